# revision 19
# baseline (speedup 1.0000x reference)
"""Trainium2 Bass kernel for nn_Attention_43542378447097.

GroupNorm -> multi-head causal self-attention -> out-proj, then the
reference's broadcast add:

    out(B,S,C) + residual(B,C,1,C)  ->  (B,C,S,C)   [right-aligned numpy
    broadcasting, so batches MIX]:

    result[i, j, k, l] = A[j, k, l] + xn[i, j, l]

where A[j] = attention output (incl bo) of batch j and xn[i] = groupnorm
output of batch i.  Output is (96, 96, 96, 96) fp32 (~340 MB) -> memory
bound; ~42.5 MB written per core.

Sharding: core c owns batches/rows i in [12c, 12c+12).
  Phase 1: groupnorm (batched over all 12 local batches) + attention in
    bf16 (tolerance is 2e-2; bf16 matmuls with fp32 PSUM accumulate are
    ~1e-3), software-pipelined in 3 chunks of 4 batches -> A_local.
  Phase 2: AllGather A_local over 8 cores -> A_full (96,96,96), ~3.5 MB.
  Phase 3 (per local i): result[i] = A_full + (xn_i + bo_eff) broadcast
    over k -- 24 half-slabs [96, 4608]; within each, DVE assembles 32
    k-rows and GpSimd 16 (fp32 tensor_tensor is 1-port, never contends
    with GpSimd), one 1.77 MB output DMA per half-slab.

Attention layout avoids cross-partition broadcasts:
  qT/kT per head via lhsT=W-slice (bias as 97th contraction row paired
  with a ones row on xnT), v natural via lhsT=xnT;  scoresT = kT.T @ qT
  (4 heads packed per PSUM bank) -> exp on ACT (bf16 out) -> causal mask
  multiply -> denominators via ones-matmul (replicated across
  partitions) -> reciprocal on DVE -> normalization folded into the AV
  eviction (PSUM * recip -> bf16) -> out = sum_h ocatT_h.T @ Wo_h.
1/sqrt(dk) folded into Wq/bq on host; bv folded into bo_eff = bv@Wo+bo
(softmax rows sum to 1); groupnorm rstd is an all-DVE Newton rsqrt and
all ACT functions (Copy/Exp/Square) share one table set -> exactly one
ACT table load.
"""

import sys

sys.path.insert(0, "/opt/trn_rl_repo")

import numpy as np

B_TOTAL = 96
C = 96
S = 96
NH = 8
DK = 96
G = 8
NCORES = 8
BPC = B_TOTAL // NCORES  # 12
EPS = 1e-5
NFREE = S * C  # 9216
HALFN = NFREE // 2  # 4608
KH = S // 2  # 48 k-rows per half-slab
DVE_K = 32  # k-rows of each half-slab assembled on DVE; rest on GpSimd
NCH = 3  # attention chunks
CB = BPC // NCH  # batches per chunk (4)

_PROG = None


def _build_program(skip_collective=False, loop_n=1, phases="123"):
    import contextlib

    import concourse.bass as bass
    import concourse.tile as tile
    from concourse import bacc, mybir

    f32 = mybir.dt.float32
    bf16 = mybir.dt.bfloat16
    i32 = mybir.dt.int32
    AF = mybir.ActivationFunctionType
    ALU = mybir.AluOpType
    AX = mybir.AxisListType

    nc = bacc.Bacc(
        "TRN2",
        target_bir_lowering=False,
        debug=False,
        enable_asserts=False,
        num_devices=NCORES,
    )

    x_d = nc.declare_dram_parameter("x", [BPC, C, C], f32, isOutput=False)
    # wq/wk carry the bias as a 97th contraction row (paired with a ones row
    # appended to xnT); wq pre-scaled by 1/sqrt(dk) on host.
    wq_d = nc.declare_dram_parameter("wq", [C + 1, NH, DK], bf16, isOutput=False)
    wk_d = nc.declare_dram_parameter("wk", [C + 1, NH, DK], bf16, isOutput=False)
    wv_d = nc.declare_dram_parameter("wv", [C, NH, DK], bf16, isOutput=False)
    wo_d = nc.declare_dram_parameter("wo", [DK, NH, C], bf16, isOutput=False)
    gamma_d = nc.declare_dram_parameter("gamma", [C, 1], f32, isOutput=False)
    beta_d = nc.declare_dram_parameter("beta", [C, 1], f32, isOutput=False)
    gmask_d = nc.declare_dram_parameter("gmask", [C, C], f32, isOutput=False)
    ones_d = nc.declare_dram_parameter("ones96", [S, S], bf16, isOutput=False)
    maskt_d = nc.declare_dram_parameter("maskT", [S, S], bf16, isOutput=False)
    iden_d = nc.declare_dram_parameter("iden", [C, C], f32, isOutput=False)
    boe_d = nc.declare_dram_parameter("bo_eff", [1, C], f32, isOutput=False)
    out_d = nc.declare_dram_parameter("out", [BPC, C, NFREE], f32, isOutput=True)

    with tile.TileContext(nc) as tc:
        with (
            tc.tile_pool(name="const", bufs=1) as cpool,
            tc.tile_pool(name="work", bufs=2) as work,
            tc.tile_pool(name="psum", bufs=1, space="PSUM") as pp,
            tc.tile_pool(name="dram", bufs=1, space="DRAM") as dpool,
        ):
            # ---- constants ----
            wq_sb = cpool.tile([C + 1, NH, DK], bf16, name="wq_sb")
            wk_sb = cpool.tile([C + 1, NH, DK], bf16, name="wk_sb")
            wv_sb = cpool.tile([C, NH, DK], bf16, name="wv_sb")
            wo_sb = cpool.tile([DK, NH, C], bf16, name="wo_sb")
            gamma_sb = cpool.tile([C, 1], f32, name="gamma_sb")
            beta_sb = cpool.tile([C, 1], f32, name="beta_sb")
            gmask_sb = cpool.tile([C, C], f32, name="gmask_sb")
            ones_sb = cpool.tile([S, S], bf16, name="ones_sb")
            maskt_sb = cpool.tile([S, S], bf16, name="maskt_sb")
            iden_sb = cpool.tile([C, C], f32, name="iden_sb")
            bo_rep = cpool.tile([C, C], f32, name="bo_rep")
            xnp_all = cpool.tile([C, BPC, C], f32, name="xnp_all")
            xnT_bf = cpool.tile([C + 1, BPC, C], bf16, name="xnT_bf")
            a_sb = cpool.tile([C, NFREE], f32, name="a_sb")

            nc.sync.dma_start(out=wq_sb, in_=wq_d[:])
            nc.sync.dma_start(out=wk_sb, in_=wk_d[:])
            nc.sync.dma_start(out=wv_sb, in_=wv_d[:])
            nc.sync.dma_start(out=wo_sb, in_=wo_d[:])
            nc.sync.dma_start(out=gamma_sb, in_=gamma_d[:])
            nc.sync.dma_start(out=beta_sb, in_=beta_d[:])
            nc.sync.dma_start(out=gmask_sb, in_=gmask_d[:])
            nc.sync.dma_start(out=ones_sb, in_=ones_d[:])
            nc.sync.dma_start(out=maskt_sb, in_=maskt_d[:])
            nc.sync.dma_start(out=iden_sb, in_=iden_d[:])
            nc.sync.dma_start(out=bo_rep, in_=boe_d[:].to_broadcast((C, C)))

            # DRAM bounce buffers for the collective
            a_loc = dpool.tile([BPC, S, C], f32, name="a_loc")
            a_full = dpool.tile(
                [NCORES * BPC, S, C],
                f32,
                name="a_full",
                addr_space="Local" if skip_collective else "Shared",
            )

            inv_n = 1.0 / (C * C // G)  # 1/1152

            loop_cm = (
                tc.For_i(0, loop_n, 1)
                if loop_n > 1
                else contextlib.nullcontext()
            )
            loop_cm.__enter__()

            # ===== phase 1: groupnorm (all batches at once) + attention
            # (3 chunks of 4 batches, software-pipelined emission).
            def gn_all():
                # PE warmup: dummy matmuls during the DVE-bound groupnorm so
                # HAM un-throttles before the real attention matmuls.
                ps_w = pp.tile([S, S], f32, tag="ps_small", bufs=1, name="ps_warm")
                for _ in range(24):
                    nc.tensor.matmul(
                        ps_w, lhsT=gmask_sb, rhs=gmask_sb, start=True, stop=True
                    )

                x_all = work.tile([C, BPC, C], f32, tag="x_all", bufs=1, name="x_all")
                nc.sync.dma_start(
                    out=x_all, in_=x_d[:].rearrange("b s c -> s b c")
                )
                s12 = work.tile([C, 2 * BPC], f32, tag="st", bufs=1, name="s12")
                nc.vector.tensor_reduce(
                    out=s12[:, 0:BPC], in_=x_all, axis=AX.X, op=ALU.add
                )
                # sum of squares via ACT Square + accumulator (no x^2 buffer)
                for b in range(BPC):
                    x2_sc = work.tile([C, C], f32, tag="x2_sc", bufs=2, name="x2_sc")
                    nc.scalar.activation(
                        out=x2_sc,
                        in_=x_all[:, b, :],
                        func=AF.Square,
                        accum_out=s12[:, BPC + b : BPC + b + 1],
                    )
                ps_g = pp.tile([C, 2 * BPC], f32, tag="ps_small", bufs=1, name="ps_g")
                nc.tensor.matmul(ps_g, lhsT=gmask_sb, rhs=s12, start=True, stop=True)
                mu = work.tile([C, BPC], f32, tag="st2", bufs=8, name="mu")
                ex2 = work.tile([C, BPC], f32, tag="st2", bufs=8, name="ex2")
                nc.vector.tensor_scalar_mul(mu, ps_g[:, 0:BPC], inv_n)
                nc.vector.tensor_scalar_mul(ex2, ps_g[:, BPC : 2 * BPC], inv_n)
                musq = work.tile([C, BPC], f32, tag="st2", bufs=8, name="musq")
                nc.vector.tensor_mul(musq, mu, mu)
                veps = work.tile([C, BPC], f32, tag="st2", bufs=8, name="veps")
                nc.vector.scalar_tensor_tensor(
                    veps, ex2, EPS, musq, op0=ALU.add, op1=ALU.subtract
                )
                # rstd = rsqrt(veps), all-DVE (quake seed + 2 Newton steps)
                iv = veps.bitcast(i32)
                ineg = work.tile([C, BPC], i32, tag="sti", bufs=8, name="ineg")
                nc.vector.tensor_scalar_mul(ineg, iv, -1)
                nc.vector.tensor_scalar(ineg, ineg, 1, None, op0=ALU.arith_shift_right)
                nc.vector.tensor_scalar(ineg, ineg, 0x5F3759DF, None, op0=ALU.add)
                y = ineg.bitcast(f32)
                t1 = work.tile([C, BPC], f32, tag="st2", bufs=8, name="t1")
                for _ in range(2):
                    nc.vector.tensor_mul(t1, y, y)
                    nc.vector.tensor_mul(t1, t1, veps)
                    nc.vector.tensor_scalar(t1, t1, -0.5, 1.5, op0=ALU.mult, op1=ALU.add)
                    nc.vector.tensor_mul(y, y, t1)
                scale_t = work.tile([C, BPC], f32, tag="st2", bufs=8, name="scale_t")
                nc.vector.tensor_tensor(
                    scale_t, y, gamma_sb.to_broadcast((C, BPC)), ALU.mult
                )
                mus = work.tile([C, BPC], f32, tag="st2", bufs=8, name="mus")
                nc.vector.tensor_mul(mus, mu, scale_t)
                shift_t = work.tile([C, BPC], f32, tag="st2", bufs=8, name="shift_t")
                nc.vector.tensor_tensor(
                    shift_t, beta_sb.to_broadcast((C, BPC)), mus, ALU.subtract
                )
                # xn = x*scale + shift, in place on x_all
                xn_all = x_all
                nc.vector.tensor_tensor(
                    xn_all,
                    x_all,
                    scale_t.unsqueeze(2).to_broadcast((C, BPC, C)),
                    ALU.mult,
                )
                nc.vector.tensor_tensor(
                    xn_all,
                    xn_all,
                    shift_t.unsqueeze(2).to_broadcast((C, BPC, C)),
                    ALU.add,
                )
                # phase-3 addend (fp32) on GpSimd
                nc.gpsimd.tensor_tensor(
                    xnp_all,
                    xn_all,
                    bo_rep.unsqueeze(1).to_broadcast((C, BPC, C)),
                    ALU.add,
                )
                nc.vector.memset(xnT_bf[C : C + 1, :, :], 1.0)
                for b in range(BPC):
                    ps_t = pp.tile([C, C], f32, tag="ps2b", bufs=3, name="ps_t")
                    nc.tensor.transpose(ps_t, xn_all[:, b, :], iden_sb)
                    nc.scalar.activation(
                        out=xnT_bf[0:C, b, :], in_=ps_t, func=AF.Copy
                    )

            qk_tiles = {}
            v_tiles = {}
            exp_tiles = {}

            def st1(c):
                """q/k/v projections for chunk c (batches 4c..4c+3)."""
                xnT_c = xnT_bf[:, CB * c : CB * (c + 1), :].rearrange(
                    "p b s -> p (b s)"
                )
                qT_c = work.tile([DK, NH, CB, S], bf16, tag="qT", bufs=2, name="qT_c")
                kT_c = work.tile([DK, NH, CB, S], bf16, tag="kT", bufs=2, name="kT_c")
                v_c = work.tile([S, CB, NH, DK], bf16, tag="v", bufs=2, name="v_c")
                qk_tiles[c] = (qT_c, kT_c)
                v_tiles[c] = v_c
                for wt, dst in ((wq_sb, qT_c), (wk_sb, kT_c)):
                    for hp in range(0, NH, 2):
                        psp = pp.tile([DK, 1024], f32, tag="ps2b", bufs=3, name="ps_qk")
                        for hh in range(2):
                            nc.tensor.matmul(
                                psp[:, hh * 512 : hh * 512 + CB * S],
                                lhsT=wt[:, hp + hh, :],
                                rhs=xnT_c,
                                start=True,
                                stop=True,
                            )
                        pv = psp.rearrange("p (g x) -> p g x", g=2)[
                            :, :, 0 : CB * S
                        ].rearrange("p g (b s) -> p g b s", b=CB)
                        nc.scalar.activation(
                            out=dst[:, hp : hp + 2], in_=pv, func=AF.Copy
                        )
                for bb in range(CB):
                    b = CB * c + bb
                    psv = pp.tile([S, 1024], f32, tag="ps2b", bufs=3, name="ps_v")
                    for half in range(2):
                        nc.tensor.matmul(
                            psv[:, half * 512 : half * 512 + 4 * DK],
                            lhsT=xnT_bf[0:C, b, :],
                            rhs=wv_sb[:, 4 * half : 4 * (half + 1), :].rearrange(
                                "p h d -> p (h d)"
                            ),
                            start=True,
                            stop=True,
                        )
                    pvv = psv.rearrange("p (g x) -> p g x", g=2)[
                        :, :, 0 : 4 * DK
                    ].rearrange("p g (h d) -> p g h d", h=4)
                    nc.scalar.activation(
                        out=v_c[:, bb].rearrange("p (g h) d -> p g h d", g=2),
                        in_=pvv,
                        func=AF.Copy,
                    )

            def st2(c):
                """scoresT -> exp -> causal mask for chunk c."""
                qT_c, kT_c = qk_tiles[c]
                expT_c = work.tile(
                    [S, CB, NH, S], bf16, tag="expT", bufs=2, name="expT_c"
                )
                exp_tiles[c] = expT_c
                for bb in range(CB):
                    ps_sc = pp.tile([S, 1024], f32, tag="ps2b", bufs=3, name="ps_sc")
                    for h in range(NH):
                        off = (h // 4) * 512 + (h % 4) * S
                        nc.tensor.matmul(
                            ps_sc[:, off : off + S],
                            lhsT=kT_c[:, h, bb, :],
                            rhs=qT_c[:, h, bb, :],
                            start=True,
                            stop=True,
                        )
                    scv = ps_sc.rearrange("p (g x) -> p g x", g=2)[
                        :, :, 0 : 4 * S
                    ].rearrange("p g (h s) -> p g h s", h=4)
                    nc.scalar.activation(
                        out=expT_c[:, bb].rearrange("p (g h) s -> p g h s", g=2),
                        in_=scv,
                        func=AF.Exp,
                    )
                    nc.gpsimd.tensor_tensor(
                        expT_c[:, bb],
                        expT_c[:, bb],
                        maskt_sb.unsqueeze(1).to_broadcast((S, NH, S)),
                        ALU.mult,
                    )

            recip_tiles = {}

            def st3(c):
                """softmax denominators + reciprocal for chunk c."""
                rhs_all = exp_tiles[c].rearrange("p b h s -> p (b h s)")
                recip_c = work.tile(
                    [S, CB * NH * S], f32, tag="recip", bufs=2, name="recip_c"
                )
                recip_tiles[c] = recip_c
                for seg in range(3):
                    psd = pp.tile([S, 1024], f32, tag="ps2b", bufs=3, name="ps_d")
                    for q2 in range(2):
                        nc.tensor.matmul(
                            psd[:, q2 * 512 : (q2 + 1) * 512],
                            lhsT=ones_sb,
                            rhs=rhs_all[:, seg * 1024 + q2 * 512 : seg * 1024 + (q2 + 1) * 512],
                            start=True,
                            stop=True,
                        )
                    nc.vector.reciprocal(
                        out=recip_c[:, seg * 1024 : (seg + 1) * 1024], in_=psd
                    )

            def st4(c):
                """AV (normalize folded into eviction) + out-proj for chunk c."""
                recip_c = recip_tiles.pop(c).rearrange(
                    "p (b h s) -> p b h s", b=CB, h=NH
                )
                expT_c = exp_tiles.pop(c)
                v_c = v_tiles.pop(c)
                ocatT_c = work.tile(
                    [DK, CB, NH, S], bf16, tag="ocatT", bufs=2, name="ocatT_c"
                )
                for bb in range(CB):
                    b = CB * c + bb
                    ps_o = pp.tile([DK, 1024], f32, tag="ps2b", bufs=3, name="ps_o")
                    for h in range(NH):
                        off = (h // 4) * 512 + (h % 4) * S
                        nc.tensor.matmul(
                            ps_o[:, off : off + S],
                            lhsT=v_c[:, bb, h, :],
                            rhs=expT_c[:, bb, h, :],
                            start=True,
                            stop=True,
                        )
                    ov = ps_o.rearrange("p (g x) -> p g x", g=2)[
                        :, :, 0 : 4 * S
                    ].rearrange("p g (h s) -> p g h s", h=4)
                    nc.vector.tensor_tensor(
                        ocatT_c[:, bb].rearrange("p (g h) s -> p g h s", g=2),
                        ov,
                        recip_c[:, bb].rearrange("p (g h) s -> p g h s", g=2),
                        ALU.mult,
                    )
                    psw = pp.tile([S, C], f32, tag="psw", bufs=1, name="ps_w")
                    for h in range(NH):
                        nc.tensor.matmul(
                            psw,
                            lhsT=ocatT_c[:, bb, h, :],
                            rhs=wo_sb[:, h, :],
                            start=(h == 0),
                            stop=(h == NH - 1),
                        )
                    outp_sb = work.tile([S, C], f32, tag="outp", bufs=2, name="outp_sb")
                    nc.scalar.activation(out=outp_sb, in_=psw, func=AF.Copy)
                    nc.sync.dma_start(out=a_loc[b], in_=outp_sb)

            if "1" in phases:
                gn_all()
                # software-pipelined emission; order keeps every reused pool
                # slot's writer emitted after that slot's previous readers on
                # each in-order engine stream.
                st1(0)
                st1(1)
                st2(0)
                st3(0)
                st2(1)
                st4(0)
                st1(2)
                st3(1)
                st2(2)
                st4(1)
                st3(2)
                st4(2)
            else:
                nc.vector.memset(xnp_all, 0.0)

            # ================= phase 2: all-gather attention outputs =======
            if "2" not in phases:
                pass
            elif skip_collective:
                # timeline-sim variant: approximate the collective's DMA cost
                for cc in range(NCORES):
                    nc.sync.dma_start(
                        out=a_full[cc * BPC : (cc + 1) * BPC].rearrange(
                            "b k l -> (b k l)"
                        ),
                        in_=a_loc[:].rearrange("b k l -> (b k l)"),
                    )
            else:
                nc.gpsimd.collective_compute(
                    "AllGather",
                    mybir.AluOpType.bypass,
                    replica_groups=[list(range(NCORES))],
                    ins=[a_loc.opt()],
                    outs=[a_full.opt()],
                )
            if "3" in phases:
                # load in k-halves so half-0 assembly overlaps the second DMA
                a_flat = a_full[:].rearrange("j k l -> j (k l)")
                qn = NFREE // 4
                for q4 in range(4):
                    nc.sync.dma_start(
                        out=a_sb[:, q4 * qn : (q4 + 1) * qn],
                        in_=a_flat[:, q4 * qn : (q4 + 1) * qn],
                    )
            a_3d = a_sb.rearrange("p (k l) -> p k l", l=C)

            # ================= phase 3: assemble + write output ============
            # 24 half-slabs; DVE takes 32 of the 48 k-rows, GpSimd 16 -- the
            # two streams run concurrently against the output DMA.
            for i in range(BPC) if "3" in phases else []:
                xnp_b = xnp_all[:, i, :].unsqueeze(1)
                for half in range(2):
                    k0 = half * KH
                    res_t = work.tile([C, HALFN], f32, tag="res", bufs=3, name="res_t")
                    res_3d = res_t.rearrange("p (k l) -> p k l", l=C)
                    nc.vector.tensor_tensor(
                        res_3d[:, 0:DVE_K, :],
                        a_3d[:, k0 : k0 + DVE_K, :],
                        xnp_b.to_broadcast((C, DVE_K, C)),
                        ALU.add,
                    )
                    nc.gpsimd.tensor_tensor(
                        res_3d[:, DVE_K:KH, :],
                        a_3d[:, k0 + DVE_K : k0 + KH, :],
                        xnp_b.to_broadcast((C, KH - DVE_K, C)),
                        ALU.add,
                    )
                    nc.sync.dma_start(
                        out=out_d[i][:, half * HALFN : (half + 1) * HALFN],
                        in_=res_t,
                    )

            loop_cm.__exit__(None, None, None)

    nc.compile()
    return nc


def _get_program():
    global _PROG
    if _PROG is None:
        _PROG = _build_program()
    return _PROG


def _host_inputs(x, Wq, bq, Wk, bk, Wv, bv, Wo, bo, gamma, beta):
    import ml_dtypes

    f32 = np.float32
    bf16 = ml_dtypes.bfloat16
    x = np.asarray(x, f32)
    Wq = np.asarray(Wq, f32)
    bq = np.asarray(bq, f32)
    Wk = np.asarray(Wk, f32)
    bk = np.asarray(bk, f32)
    Wv = np.asarray(Wv, f32)
    bv = np.asarray(bv, f32)
    Wo = np.asarray(Wo, f32)
    bo = np.asarray(bo, f32)
    gamma = np.asarray(gamma, f32)
    beta = np.asarray(beta, f32)

    sc = f32(1.0 / np.sqrt(DK))
    wq97 = np.concatenate(
        [(Wq * sc).reshape(C, NH, DK), (bq * sc).reshape(1, NH, DK)], axis=0
    )
    wk97 = np.concatenate(
        [Wk.reshape(C, NH, DK), bk.reshape(1, NH, DK)], axis=0
    )
    com = {
        "wq": np.ascontiguousarray(wq97).astype(bf16),
        "wk": np.ascontiguousarray(wk97).astype(bf16),
        "wv": np.ascontiguousarray(Wv.reshape(C, NH, DK)).astype(bf16),
        "wo": np.ascontiguousarray(
            Wo.reshape(NH, DK, C).transpose(1, 0, 2)
        ).astype(bf16),
        "gamma": np.ascontiguousarray(gamma.reshape(C, 1)),
        "beta": np.ascontiguousarray(beta.reshape(C, 1)),
        "gmask": np.kron(np.eye(G, dtype=f32), np.ones((C // G, C // G), f32)),
        "ones96": np.ones((S, S), bf16),
        "maskT": np.triu(np.ones((S, S), f32)).astype(bf16),
        "iden": np.eye(C, dtype=f32),
        "bo_eff": (bv.astype(np.float64) @ Wo.astype(np.float64) + bo)
        .astype(f32)
        .reshape(1, C),
    }
    x_r = np.ascontiguousarray(x.reshape(B_TOTAL, C, C))
    in_maps = []
    for i in range(NCORES):
        m = dict(com)
        m["x"] = np.ascontiguousarray(x_r[i * BPC : (i + 1) * BPC])
        in_maps.append(m)
    return in_maps


def _run(inputs, trace=False):
    from concourse.bass_utils import run_bass_kernel_spmd

    nc = _get_program()
    in_maps = _host_inputs(**inputs)
    res = run_bass_kernel_spmd(
        nc, in_maps, core_ids=list(range(NCORES)), trace=trace
    )
    out = np.concatenate([r["out"] for r in res.results], axis=0)
    return out.reshape(B_TOTAL, C, S, C).astype(np.float32), res


def kernel(**inputs) -> np.ndarray:
    out, _ = _run(inputs, trace=False)
    return out
